# revision 1
# baseline (speedup 1.0000x reference)
"""Contrastive-loss Trainium2 kernel: 8-way data-parallel over similarity rows.

Strategy (per sharding hint): each of the 8 NeuronCores computes a
[1024, 8192] block of the similarity matrix sim = e @ e.T / T against the
full embedding matrix, reduces per-row numerator / denominator / validity
on-device, and returns per-partition partial (loss_sum, valid_count); the
host sums the 8x[128,2] partials.

Key layout trick: rows are sorted by label on the host and each core's
input is rolled so its 1024 rows sit at a fixed offset (PAD). Same-label
columns of any 128-row tile then live in a fixed 640-wide window
[t*128, t*128+640), so the label-mask / positive-gate / numerator work
touches 640 instead of 8192 columns per row. The denominator row-sum comes
free from the Exp activation's accum_out. Matmuls run in bf16 (fp32 PSUM
accumulate); everything downstream of exp is fp32.
"""

import contextlib
import ctypes
import os
import sys
import types

import ml_dtypes
import numpy as np

import concourse.bass as bass
import concourse.mybir as mybir
import concourse.tile as tile
from concourse.bass_utils import run_bass_kernel_spmd

# problem constants (hardcoded per task contract)
N, D, NCLS = 8192, 512, 512
TEMP = 0.07
EPS = 1e-8
M = 8            # cores
R = N // M       # 1024 rows per core
NT = R // 128    # 8 row-tiles per core
PAD = 256        # roll margin; must exceed max class size
WIN = 128 + 2 * PAD   # 640 col window containing all same-label cols of a tile
CH = 512         # matmul moving-dim chunk (one PSUM bank)
GRP = 2048       # columns per psum group / exp call (4 banks)
NG = N // GRP    # 4 groups
KT = D // 128    # 4 contraction tiles

_AXON_SO = "/opt/axon/libaxon_pjrt.so"

LAST_RESULTS = None   # BassKernelResults of the most recent run (for test.py)


def _install_axon_trace_hook():
    """Provide antenv.axon_hooks (NTFF profiling) if the image lacks it."""
    try:
        from antenv.axon_hooks import get_axon_ntff_profile_hook  # noqa: F401
        return
    except ImportError:
        pass
    if not os.path.exists(_AXON_SO):
        return
    try:
        lib = ctypes.CDLL(_AXON_SO)
    except OSError:
        return
    if not hasattr(lib, "axon_start_nrt_profile"):
        return
    lib.axon_start_nrt_profile.argtypes = [ctypes.POINTER(ctypes.c_int64), ctypes.c_size_t]
    lib.axon_start_nrt_profile.restype = ctypes.c_int64
    lib.axon_stop_nrt_profile.argtypes = [ctypes.c_char_p]
    lib.axon_stop_nrt_profile.restype = ctypes.c_int64

    @contextlib.contextmanager
    def _hook(output_dir, device_ids):
        import jax
        jax.devices()
        if device_ids:
            ids = (ctypes.c_int64 * len(device_ids))(*device_ids)
            rc = lib.axon_start_nrt_profile(ids, len(device_ids))
        else:
            rc = lib.axon_start_nrt_profile(None, 0)
        if rc != 0:
            raise RuntimeError(f"axon_start_nrt_profile rc={rc}")
        try:
            yield
        finally:
            n = lib.axon_stop_nrt_profile(str(output_dir).encode())
            if n < 0:
                raise RuntimeError(f"axon_stop_nrt_profile rc={n}")

    _the_hook = [_hook]
    mod = types.ModuleType("antenv.axon_hooks")
    mod.set_axon_ntff_profile_hook = lambda h: _the_hook.__setitem__(0, h)
    mod.get_axon_ntff_profile_hook = lambda: _the_hook[0]
    sys.modules["antenv.axon_hooks"] = mod
    import antenv
    antenv.axon_hooks = mod


def _split_excess_waits(nc, max_waits=1):
    """This walrus build allows one sync-wait per instruction; move extras
    onto same-engine NoOps inserted just before (execution order preserved)."""
    for f in nc.m.functions:
        for b in f.blocks:
            insts = b.instructions
            new = []
            changed = False
            for inst in insts:
                si = inst.sync_info
                ow = list(si.on_wait) if (si and si.on_wait) else []
                if len(ow) > max_waits:
                    extra, keep = ow[:-max_waits], ow[-max_waits:]
                    for k, w in enumerate(extra):
                        nop = mybir.InstNoOp(name=f"{inst.name}-w{k}", ins=[], outs=[])
                        nop.engine = inst.engine
                        nop.sync_info = mybir.SyncInfo(on_wait=[w], on_update=[])
                        new.append(nop)
                    inst.sync_info = mybir.SyncInfo(
                        on_wait=keep,
                        on_update=list(si.on_update) if si.on_update else [])
                    changed = True
                new.append(inst)
            if changed:
                b.instructions = new


def _build_nc():
    f32 = mybir.dt.float32
    bf16 = mybir.dt.bfloat16
    Alu = mybir.AluOpType
    Act = mybir.ActivationFunctionType

    nc = bass.Bass(trn_type="TRN2", target_bir_lowering=False, debug=False)
    qT = nc.dram_tensor("qT", [128, (N // CH) * KT * CH], bf16, kind="ExternalInput")
    labd = nc.dram_tensor("lab", [N, 1], f32, kind="ExternalInput")
    antid = nc.dram_tensor("anti", [128, 128], f32, kind="ExternalInput")
    identd = nc.dram_tensor("ident", [128, 128], f32, kind="ExternalInput")
    outd = nc.dram_tensor("out", [128, 2], f32, kind="ExternalOutput")

    with tile.TileContext(nc) as tc, contextlib.ExitStack() as ctx:
        qp = ctx.enter_context(tc.tile_pool(name="qp", bufs=1))
        pp = ctx.enter_context(tc.tile_pool(name="pp", bufs=2, space="PSUM"))
        ep = ctx.enter_context(tc.tile_pool(name="ep", bufs=3))
        wp = ctx.enter_context(tc.tile_pool(name="wp", bufs=2))
        sp = ctx.enter_context(tc.tile_pool(name="sp", bufs=1))

        # ---- preload ----
        # qT chunks: [128, KT, CH] bf16, one per 512-col chunk
        qt = []
        for n in range(N // CH):
            q = qp.tile([128, KT, CH], bf16, tag=f"q{n}")
            nc.sync.dma_start(
                out=q, in_=qT[:, n * KT * CH:(n + 1) * KT * CH])
            qt.append(q)
        # row labels per (partition, tile): lab[PAD + t*128 + p]
        lab_rows = sp.tile([128, NT, 1], f32)
        nc.sync.dma_start(
            out=lab_rows,
            in_=labd[PAD:PAD + R, :].rearrange("(t p) o -> p t o", p=128))
        # column labels broadcast to all partitions, cols [0, NT*128+WIN)
        labw_w = (NT - 1) * 128 + WIN        # 1536
        labw = sp.tile([128, labw_w], f32)
        nc.sync.dma_start(
            out=labw,
            in_=bass.AP(tensor=labd, offset=0, ap=[[0, 128], [1, labw_w]]))
        anti = sp.tile([128, 128], f32)
        nc.sync.dma_start(out=anti, in_=antid.ap())
        ident = sp.tile([128, 128], f32)
        nc.sync.dma_start(out=ident, in_=identd.ap())
        eps_t = sp.tile([128, 1], f32)
        nc.vector.memset(eps_t, EPS)
        warm = sp.tile([128, 128], bf16)
        nc.vector.memset(warm, 0.0)
        warm_ps = pp.tile([128, GRP], f32, tag="ps")
        for w in range(48):
            nc.tensor.matmul(warm_ps[:, :128], warm, warm, start=True, stop=True)

        # ---- accumulators ----
        dacc = sp.tile([128, NT * NG], f32)   # exp row-sums per (t, g)
        nacc = sp.tile([128, NT], f32)        # numerator per t
        edacc = sp.tile([128, NT], f32)       # diagonal exp per t

        # ---- main loop ----
        for t in range(NT):
            a = (PAD + t * 128) // CH        # lhsT chunk index
            off = (PAD + t * 128) % CH       # lhsT offset within chunk
            for g in range(NG):
                ps = pp.tile([128, GRP], f32, tag="ps")
                for sub in range(GRP // CH):
                    n = g * (GRP // CH) + sub
                    for k in range(KT):
                        nc.tensor.matmul(
                            ps[:, sub * CH:(sub + 1) * CH],
                            qt[a][:, k, off:off + 128],
                            qt[n][:, k, :],
                            start=(k == 0), stop=(k == KT - 1))
                e = ep.tile([128, GRP], f32, tag="e")
                nc.scalar.activation(
                    out=e, in_=ps[:], func=Act.Exp, scale=float(1.0 / TEMP),
                    accum_out=dacc[:, t * NG + g:t * NG + g + 1])
                if g == 0:
                    # window = cols [t*128, t*128+WIN) -- inside group 0
                    w0 = t * 128
                    u = wp.tile([128, WIN], f32, tag="u")
                    # u = (lab_col == lab_row) * exp(sim)
                    nc.vector.scalar_tensor_tensor(
                        out=u, in0=labw[:, w0:w0 + WIN],
                        scalar=lab_rows[:, t, :], in1=e[:, w0:w0 + WIN],
                        op0=Alu.is_equal, op1=Alu.mult)
                    # diagonal sits at window cols [PAD, PAD+128)
                    scr = wp.tile([128, 128], f32, tag="scr")
                    nc.vector.scalar_tensor_tensor(
                        out=scr, in0=u[:, PAD:PAD + 128], scalar=1.0,
                        in1=ident, op0=Alu.mult, op1=Alu.mult,
                        accum_out=edacc[:, t:t + 1])
                    nc.vector.tensor_tensor(
                        out=u[:, PAD:PAD + 128], in0=u[:, PAD:PAD + 128],
                        in1=anti, op=Alu.mult)
                    # numerator: sum over u where u > 1  (sim>0 gate)
                    scr2 = wp.tile([128, WIN], f32, tag="scr2")
                    nc.vector.scalar_tensor_tensor(
                        out=scr2, in0=u, scalar=1.0, in1=u,
                        op0=Alu.is_gt, op1=Alu.mult,
                        accum_out=nacc[:, t:t + 1])

        # ---- epilogue (all [128, NT]) ----
        dred = sp.tile([128, NT], f32)
        nc.vector.tensor_reduce(
            out=dred, in_=dacc.rearrange("p (t g) -> p t g", g=NG),
            axis=mybir.AxisListType.X, op=Alu.add)
        den = sp.tile([128, NT], f32)
        nc.vector.tensor_tensor(out=den, in0=dred, in1=edacc, op=Alu.subtract)
        v1 = sp.tile([128, NT], f32)
        nc.vector.tensor_scalar(out=v1, in0=nacc, scalar1=0.0, scalar2=None,
                                op0=Alu.is_gt)
        v2 = sp.tile([128, NT], f32)
        nc.vector.tensor_scalar(out=v2, in0=den, scalar1=0.0, scalar2=None,
                                op0=Alu.is_gt)
        v = sp.tile([128, NT], f32)
        nc.vector.tensor_tensor(out=v, in0=v1, in1=v2, op=Alu.mult)
        inv = sp.tile([128, NT], f32)
        nc.vector.tensor_scalar(out=inv, in0=v, scalar1=0.0, scalar2=None,
                                op0=Alu.is_equal)
        nsafe = sp.tile([128, NT], f32)
        nc.vector.tensor_tensor(out=nsafe, in0=nacc, in1=v, op=Alu.mult)
        nc.vector.tensor_tensor(out=nsafe, in0=nsafe, in1=inv, op=Alu.add)
        dsafe = sp.tile([128, NT], f32)
        nc.vector.tensor_tensor(out=dsafe, in0=den, in1=v, op=Alu.mult)
        nc.vector.tensor_tensor(out=dsafe, in0=dsafe, in1=inv, op=Alu.add)
        lgd = sp.tile([128, NT], f32)
        nc.scalar.activation(out=lgd, in_=dsafe, func=Act.Ln, bias=eps_t[:], scale=1.0)
        lgn = sp.tile([128, NT], f32)
        nc.scalar.activation(out=lgn, in_=nsafe, func=Act.Ln, scale=1.0)
        li = sp.tile([128, NT], f32)
        nc.vector.tensor_tensor(out=li, in0=lgd, in1=lgn, op=Alu.subtract)
        nc.vector.tensor_tensor(out=li, in0=li, in1=v, op=Alu.mult)
        o = sp.tile([128, 2], f32)
        nc.vector.tensor_reduce(out=o[:, 0:1], in_=li, axis=mybir.AxisListType.X,
                                op=Alu.add)
        nc.vector.tensor_reduce(out=o[:, 1:2], in_=v, axis=mybir.AxisListType.X,
                                op=Alu.add)
        nc.sync.dma_start(out=outd.ap(), in_=o)

    _split_excess_waits(nc)
    return nc


_NC = None


def _get_nc():
    global _NC
    if _NC is None:
        _NC = _build_nc()
    return _NC


def _host_reference(emb, lab):
    """Numpy fallback (only for pathological label distributions where a
    class exceeds the PAD margin; never triggers for the target regime)."""
    e = emb / np.linalg.norm(emb, axis=1, keepdims=True).astype(np.float32)
    sim = (e @ e.T).astype(np.float32) / np.float32(TEMP)
    E = np.exp(sim, dtype=np.float32)
    pos = (lab[:, None] == lab[None, :]) & ~np.eye(len(lab), dtype=bool)
    valid = pos & (sim > 0)
    num = np.where(valid, E, 0).sum(1, dtype=np.float32)
    den = E.sum(1, dtype=np.float32) - np.diagonal(E)
    rv = valid.any(1) & (den > 0)
    ns = np.where(rv, num, np.float32(1.0))
    ds = np.where(rv, den, np.float32(1.0))
    li = np.log(ds + np.float32(EPS)) - np.log(ns)
    nv = int(rv.sum())
    if nv == 0:
        return np.float32(0.0)
    return np.float32(abs(float(np.where(rv, li, 0).sum(dtype=np.float64)) / nv))


def kernel(**inputs):
    global LAST_RESULTS
    emb = np.ascontiguousarray(np.asarray(inputs["embeddings"], dtype=np.float32))
    lab = np.asarray(inputs["labels"]).astype(np.int64).ravel()
    assert emb.shape == (N, D) and lab.shape == (N,)

    if np.bincount(lab, minlength=1).max() > PAD:
        return _host_reference(emb, lab)

    _install_axon_trace_hook()

    # host prep: normalize, sort by label, per-core roll + transpose
    e = emb / np.linalg.norm(emb, axis=1, keepdims=True).astype(np.float32)
    order = np.argsort(lab, kind="stable")
    es = np.ascontiguousarray(e[order])
    ls = lab[order].astype(np.float32)

    anti = (1.0 - np.eye(128, dtype=np.float32)).astype(np.float32)
    ident = np.eye(128, dtype=np.float32)

    in_maps = []
    for c in range(M):
        shift = c * R - PAD
        rolled = np.roll(es, -shift, axis=0)         # [N, D] f32
        labr = np.roll(ls, -shift).reshape(N, 1)     # [N, 1] f32
        # [D, N] -> [128, NCH, KT, CH]: partition p, chunk n holds
        # qT[k*128+p, n*CH:(n+1)*CH] contiguckus per (k)
        qTc = (rolled.T.reshape(KT, 128, N // CH, CH)
               .transpose(1, 2, 0, 3)
               .reshape(128, (N // CH) * KT * CH)
               .astype(ml_dtypes.bfloat16))
        qTc = np.ascontiguousarray(qTc)
        in_maps.append({
            "qT": qTc,
            "lab": np.ascontiguousarray(labr),
            "anti": anti,
            "ident": ident,
        })

    nc = _get_nc()
    res = run_bass_kernel_spmd(nc, in_maps, core_ids=list(range(M)))
    LAST_RESULTS = res

    loss_sum = 0.0
    cnt = 0.0
    for c in range(M):
        o = res.results[c]["out"]
        loss_sum += float(o[:, 0].sum(dtype=np.float64))
        cnt += float(o[:, 1].sum(dtype=np.float64))
    if cnt <= 0:
        return np.float32(0.0)
    return np.float32(abs(loss_sum / cnt))



# revision 6
# speedup vs baseline: 1.5388x; 1.5388x over previous
"""Contrastive-loss Trainium2 kernel: 8-way data-parallel over similarity rows.

Strategy (per sharding hint): each of the 8 NeuronCores computes a
[1024, 8192] block of the similarity matrix sim = e @ e.T / T against the
full embedding matrix, reduces per-row numerator / denominator / validity
on-device, and returns per-partition partial (loss_sum, valid_count); the
host sums the 8x[128,2] partials.

Key layout trick: rows are sorted by label on the host and each core's
input is rolled so its 1024 rows sit at a fixed offset (PAD). Same-label
columns of any 128-row tile then live in a fixed 640-wide window
[t*128, t*128+640), so the label-mask / positive-gate / numerator work
touches 640 instead of 8192 columns per row. The denominator row-sum comes
free from the Exp activation's accum_out. Matmuls run in bf16 (fp32 PSUM
accumulate); everything downstream of exp is fp32.
"""

import contextlib
import ctypes
import os
import sys
import types

import ml_dtypes
import numpy as np

import concourse.bass as bass
import concourse.mybir as mybir
import concourse.tile as tile
from concourse.bass_utils import run_bass_kernel_spmd

# problem constants (hardcoded per task contract)
N, D, NCLS = 8192, 512, 512
TEMP = 0.07
EPS = 1e-8
M = 8            # cores
R = N // M       # 1024 rows per core
NT = R // 128    # 8 row-tiles per core
PAD = 256        # roll margin; must exceed max class size
WIN = 128 + 2 * PAD   # 640 col window containing all same-label cols of a tile
CH = 512         # matmul moving-dim chunk (one PSUM bank)
GRP = 2048       # columns per psum group / exp call (4 banks)
NG = N // GRP    # 4 groups
KT = D // 128    # 4 contraction tiles
FP8_SCALE = 16.0  # pre-quantization scale (keeps fp8 e4m3 out of denormals)

_AXON_SO = "/opt/axon/libaxon_pjrt.so"

LAST_RESULTS = None   # BassKernelResults of the most recent run (for test.py)


def _install_axon_trace_hook():
    """Provide antenv.axon_hooks (NTFF profiling) if the image lacks it."""
    try:
        from antenv.axon_hooks import get_axon_ntff_profile_hook  # noqa: F401
        return
    except ImportError:
        pass
    if not os.path.exists(_AXON_SO):
        return
    try:
        lib = ctypes.CDLL(_AXON_SO)
    except OSError:
        return
    if not hasattr(lib, "axon_start_nrt_profile"):
        return
    lib.axon_start_nrt_profile.argtypes = [ctypes.POINTER(ctypes.c_int64), ctypes.c_size_t]
    lib.axon_start_nrt_profile.restype = ctypes.c_int64
    lib.axon_stop_nrt_profile.argtypes = [ctypes.c_char_p]
    lib.axon_stop_nrt_profile.restype = ctypes.c_int64

    @contextlib.contextmanager
    def _hook(output_dir, device_ids):
        import jax
        jax.devices()
        if device_ids:
            ids = (ctypes.c_int64 * len(device_ids))(*device_ids)
            rc = lib.axon_start_nrt_profile(ids, len(device_ids))
        else:
            rc = lib.axon_start_nrt_profile(None, 0)
        if rc != 0:
            raise RuntimeError(f"axon_start_nrt_profile rc={rc}")
        try:
            yield
        finally:
            n = lib.axon_stop_nrt_profile(str(output_dir).encode())
            if n < 0:
                raise RuntimeError(f"axon_stop_nrt_profile rc={n}")

    _the_hook = [_hook]
    mod = types.ModuleType("antenv.axon_hooks")
    mod.set_axon_ntff_profile_hook = lambda h: _the_hook.__setitem__(0, h)
    mod.get_axon_ntff_profile_hook = lambda: _the_hook[0]
    sys.modules["antenv.axon_hooks"] = mod
    import antenv
    antenv.axon_hooks = mod


def _split_excess_waits(nc, max_waits=1):
    """This walrus build allows one sync-wait per instruction; move extras
    onto same-engine NoOps inserted just before (execution order preserved)."""
    for f in nc.m.functions:
        for b in f.blocks:
            insts = b.instructions
            new = []
            changed = False
            for inst in insts:
                si = inst.sync_info
                ow = list(si.on_wait) if (si and si.on_wait) else []
                if len(ow) > max_waits:
                    extra, keep = ow[:-max_waits], ow[-max_waits:]
                    for k, w in enumerate(extra):
                        nop = mybir.InstNoOp(name=f"{inst.name}-w{k}", ins=[], outs=[])
                        nop.engine = inst.engine
                        nop.sync_info = mybir.SyncInfo(on_wait=[w], on_update=[])
                        new.append(nop)
                    inst.sync_info = mybir.SyncInfo(
                        on_wait=keep,
                        on_update=list(si.on_update) if si.on_update else [])
                    changed = True
                new.append(inst)
            if changed:
                b.instructions = new


def _build_nc():
    f32 = mybir.dt.float32
    bf16 = mybir.dt.bfloat16
    fp8 = mybir.dt.float8e4
    Alu = mybir.AluOpType
    Act = mybir.ActivationFunctionType

    nc = bass.Bass(trn_type="TRN2", target_bir_lowering=False, debug=False)
    qT = nc.dram_tensor("qT", [128, (N // CH) * KT * CH], fp8, kind="ExternalInput")
    labd = nc.dram_tensor("lab", [N, 1], f32, kind="ExternalInput")
    antid = nc.dram_tensor("anti", [128, 128], f32, kind="ExternalInput")
    identd = nc.dram_tensor("ident", [128, 128], f32, kind="ExternalInput")
    outd = nc.dram_tensor("out", [128, 2], f32, kind="ExternalOutput")

    with tile.TileContext(nc) as tc, contextlib.ExitStack() as ctx:
        qp = ctx.enter_context(tc.tile_pool(name="qp", bufs=1))
        pp = ctx.enter_context(tc.tile_pool(name="pp", bufs=2, space="PSUM"))
        ep = ctx.enter_context(tc.tile_pool(name="ep", bufs=3))
        wp = ctx.enter_context(tc.tile_pool(name="wp", bufs=2))
        sp = ctx.enter_context(tc.tile_pool(name="sp", bufs=1))

        # ---- preload ----
        # qT chunks: [128, KT, CH] bf16, one per 512-col chunk
        qt = []
        for n in range(N // CH):
            q = qp.tile([128, KT, CH], fp8, tag=f"q{n}")
            nc.sync.dma_start(
                out=q, in_=qT[:, n * KT * CH:(n + 1) * KT * CH])
            qt.append(q)
        # row labels per (partition, tile): lab[PAD + t*128 + p]
        lab_rows = sp.tile([128, NT, 1], f32)
        nc.sync.dma_start(
            out=lab_rows,
            in_=labd[PAD:PAD + R, :].rearrange("(t p) o -> p t o", p=128))
        # column labels broadcast to all partitions, cols [0, NT*128+WIN)
        labw_w = (NT - 1) * 128 + WIN        # 1536
        labw = sp.tile([128, labw_w], f32)
        nc.sync.dma_start(
            out=labw,
            in_=bass.AP(tensor=labd, offset=0, ap=[[0, 128], [1, labw_w]]))
        anti = sp.tile([128, 128], f32)
        nc.sync.dma_start(out=anti, in_=antid.ap())
        ident = sp.tile([128, 128], f32)
        nc.sync.dma_start(out=ident, in_=identd.ap())
        eps_t = sp.tile([128, 1], f32)
        nc.vector.memset(eps_t, EPS)
        warm = sp.tile([128, 128], bf16)
        nc.vector.memset(warm, 0.0)
        warm_ps = pp.tile([128, GRP], f32, tag="ps")
        for w in range(48):
            nc.tensor.matmul(warm_ps[:, :128], warm, warm, start=True, stop=True)

        # ---- accumulators ----
        dacc = sp.tile([128, NT * NG], f32)   # exp row-sums per (t, g)
        nacc = sp.tile([128, NT], f32)        # numerator per t
        edacc = sp.tile([128, NT], f32)       # diagonal exp per t

        # ---- main loop ----
        for t in range(NT):
            a = (PAD + t * 128) // CH        # lhsT chunk index
            off = (PAD + t * 128) % CH       # lhsT offset within chunk
            for g in range(NG):
                ps = pp.tile([128, GRP], f32, tag="ps")
                for sub in range(GRP // CH):
                    n = g * (GRP // CH) + sub
                    for k in range(0, KT, 2):
                        nc.tensor.matmul(
                            ps[:, sub * CH:(sub + 1) * CH],
                            qt[a][:, k:k + 2, off:off + 128],
                            qt[n][:, k:k + 2, :],
                            start=(k == 0), stop=(k == KT - 2),
                            perf_mode=mybir.MatmulPerfMode.DoubleRow)
                e = ep.tile([128, GRP], f32, tag="e")
                nc.scalar.activation(
                    out=e, in_=ps[:], func=Act.Exp,
                    scale=float(1.0 / (TEMP * FP8_SCALE * FP8_SCALE)),
                    accum_out=dacc[:, t * NG + g:t * NG + g + 1])
                if g == 0:
                    # window = cols [t*128, t*128+WIN) -- inside group 0
                    w0 = t * 128
                    u = wp.tile([128, WIN], f32, tag="u")
                    # u = (lab_col == lab_row) * exp(sim)
                    nc.vector.scalar_tensor_tensor(
                        out=u, in0=labw[:, w0:w0 + WIN],
                        scalar=lab_rows[:, t, :], in1=e[:, w0:w0 + WIN],
                        op0=Alu.is_equal, op1=Alu.mult)
                    # diagonal sits at window cols [PAD, PAD+128)
                    scr = wp.tile([128, 128], f32, tag="scr")
                    nc.vector.scalar_tensor_tensor(
                        out=scr, in0=u[:, PAD:PAD + 128], scalar=1.0,
                        in1=ident, op0=Alu.mult, op1=Alu.mult,
                        accum_out=edacc[:, t:t + 1])
                    nc.vector.tensor_tensor(
                        out=u[:, PAD:PAD + 128], in0=u[:, PAD:PAD + 128],
                        in1=anti, op=Alu.mult)
                    # numerator: sum over u where u > 1  (sim>0 gate)
                    scr2 = wp.tile([128, WIN], f32, tag="scr2")
                    nc.vector.scalar_tensor_tensor(
                        out=scr2, in0=u, scalar=1.0, in1=u,
                        op0=Alu.is_gt, op1=Alu.mult,
                        accum_out=nacc[:, t:t + 1])

        # ---- epilogue (all [128, NT]) ----
        dred = sp.tile([128, NT], f32)
        nc.vector.tensor_reduce(
            out=dred, in_=dacc.rearrange("p (t g) -> p t g", g=NG),
            axis=mybir.AxisListType.X, op=Alu.add)
        den = sp.tile([128, NT], f32)
        nc.vector.tensor_tensor(out=den, in0=dred, in1=edacc, op=Alu.subtract)
        v1 = sp.tile([128, NT], f32)
        nc.vector.tensor_scalar(out=v1, in0=nacc, scalar1=0.0, scalar2=None,
                                op0=Alu.is_gt)
        v2 = sp.tile([128, NT], f32)
        nc.vector.tensor_scalar(out=v2, in0=den, scalar1=0.0, scalar2=None,
                                op0=Alu.is_gt)
        v = sp.tile([128, NT], f32)
        nc.vector.tensor_tensor(out=v, in0=v1, in1=v2, op=Alu.mult)
        inv = sp.tile([128, NT], f32)
        nc.vector.tensor_scalar(out=inv, in0=v, scalar1=0.0, scalar2=None,
                                op0=Alu.is_equal)
        nsafe = sp.tile([128, NT], f32)
        nc.vector.tensor_tensor(out=nsafe, in0=nacc, in1=v, op=Alu.mult)
        nc.vector.tensor_tensor(out=nsafe, in0=nsafe, in1=inv, op=Alu.add)
        dsafe = sp.tile([128, NT], f32)
        nc.vector.tensor_tensor(out=dsafe, in0=den, in1=v, op=Alu.mult)
        nc.vector.tensor_tensor(out=dsafe, in0=dsafe, in1=inv, op=Alu.add)
        lgd = sp.tile([128, NT], f32)
        nc.scalar.activation(out=lgd, in_=dsafe, func=Act.Ln, bias=eps_t[:], scale=1.0)
        lgn = sp.tile([128, NT], f32)
        nc.scalar.activation(out=lgn, in_=nsafe, func=Act.Ln, scale=1.0)
        li = sp.tile([128, NT], f32)
        nc.vector.tensor_tensor(out=li, in0=lgd, in1=lgn, op=Alu.subtract)
        nc.vector.tensor_tensor(out=li, in0=li, in1=v, op=Alu.mult)
        o = sp.tile([128, 2], f32)
        nc.vector.tensor_reduce(out=o[:, 0:1], in_=li, axis=mybir.AxisListType.X,
                                op=Alu.add)
        nc.vector.tensor_reduce(out=o[:, 1:2], in_=v, axis=mybir.AxisListType.X,
                                op=Alu.add)
        nc.sync.dma_start(out=outd.ap(), in_=o)

    _split_excess_waits(nc)
    return nc


_NC = None


def _get_nc():
    global _NC
    if _NC is None:
        _NC = _build_nc()
    return _NC


def _host_reference(emb, lab):
    """Numpy fallback (only for pathological label distributions where a
    class exceeds the PAD margin; never triggers for the target regime)."""
    e = emb / np.linalg.norm(emb, axis=1, keepdims=True).astype(np.float32)
    sim = (e @ e.T).astype(np.float32) / np.float32(TEMP)
    E = np.exp(sim, dtype=np.float32)
    pos = (lab[:, None] == lab[None, :]) & ~np.eye(len(lab), dtype=bool)
    valid = pos & (sim > 0)
    num = np.where(valid, E, 0).sum(1, dtype=np.float32)
    den = E.sum(1, dtype=np.float32) - np.diagonal(E)
    rv = valid.any(1) & (den > 0)
    ns = np.where(rv, num, np.float32(1.0))
    ds = np.where(rv, den, np.float32(1.0))
    li = np.log(ds + np.float32(EPS)) - np.log(ns)
    nv = int(rv.sum())
    if nv == 0:
        return np.float32(0.0)
    return np.float32(abs(float(np.where(rv, li, 0).sum(dtype=np.float64)) / nv))


def kernel(**inputs):
    global LAST_RESULTS
    emb = np.ascontiguousarray(np.asarray(inputs["embeddings"], dtype=np.float32))
    lab = np.asarray(inputs["labels"]).astype(np.int64).ravel()
    assert emb.shape == (N, D) and lab.shape == (N,)

    if np.bincount(lab, minlength=1).max() > PAD:
        return _host_reference(emb, lab)

    _install_axon_trace_hook()

    # host prep: normalize, sort by label, per-core roll + transpose
    e = emb / np.linalg.norm(emb, axis=1, keepdims=True).astype(np.float32)
    order = np.argsort(lab, kind="stable")
    es = np.ascontiguousarray(e[order])
    ls = lab[order].astype(np.float32)

    anti = (1.0 - np.eye(128, dtype=np.float32)).astype(np.float32)
    ident = np.eye(128, dtype=np.float32)

    in_maps = []
    for c in range(M):
        shift = c * R - PAD
        rolled = np.roll(es, -shift, axis=0)         # [N, D] f32
        labr = np.roll(ls, -shift).reshape(N, 1)     # [N, 1] f32
        # [D, N] -> [128, NCH, KT, CH]: partition p, chunk n holds
        # qT[k*128+p, n*CH:(n+1)*CH] contiguckus per (k)
        qTc = ((rolled.T * np.float32(FP8_SCALE))
               .reshape(KT, 128, N // CH, CH)
               .transpose(1, 2, 0, 3)
               .reshape(128, (N // CH) * KT * CH)
               .astype(ml_dtypes.float8_e4m3))
        qTc = np.ascontiguousarray(qTc)
        in_maps.append({
            "qT": qTc,
            "lab": np.ascontiguousarray(labr),
            "anti": anti,
            "ident": ident,
        })

    nc = _get_nc()
    res = run_bass_kernel_spmd(nc, in_maps, core_ids=list(range(M)))
    LAST_RESULTS = res

    loss_sum = 0.0
    cnt = 0.0
    for c in range(M):
        o = res.results[c]["out"]
        loss_sum += float(o[:, 0].sum(dtype=np.float64))
        cnt += float(o[:, 1].sum(dtype=np.float64))
    if cnt <= 0:
        return np.float32(0.0)
    return np.float32(abs(loss_sum / cnt))



# revision 14
# speedup vs baseline: 1.9591x; 1.2732x over previous
"""Contrastive-loss Trainium2 kernel: circulant-symmetric fp8 variant.

sim = e@e.T is symmetric: computing block (i, j) also yields block (j, i).
Each core's data is rolled by c*128 rows so one SPMD program works for all
cores: local row-tiles L in {0,8,...,56} each compute local column tiles at
circulant distance d = 0..32 (mod 64). d in [1,31] blocks contribute the
transpose side via COLUMN sums; the d=32 tile is computed by both endpoints
(row-sum only, no colsum), so nothing is double counted.

 - matmuls in fp8e4 DoubleRow mode (2x bf16 throughput), K=512, rhs 512-wide.
 - ACT exp -> bf16 scratch; the 128-wide diagonal tile is exp'd separately
   in f32 (the e^(1/T)~1.6e6 self-term must cancel exactly in den).
 - DVE accumulates scratch into per-chunk E_acc tiles (bf16, 4x mode) with
   accum_out capturing telescoped running row-sums (A-slots); the host takes
   diffs, so den row parts need no ACT accumulator reads.
 - PE column-sums each 128-wide E_acc tile (stationary weights x ones) into
   one PSUM strip; the same trick gives the masked numerator's below-diag
   band (labels sorted: positives live within +-64 rows).
 - Host: telescope diffs, un-roll, cross-core den/num assembly, log-loss.
"""

import contextlib
import ctypes
import os
import sys
import types

import ml_dtypes
import numpy as np

import concourse.bass as bass
import concourse.mybir as mybir
import concourse.tile as tile
from concourse.bass_utils import run_bass_kernel_spmd

# problem constants (hardcoded per task contract)
N, D, NCLS = 8192, 512, 512
TEMP = 0.07
EPS = 1e-8
M = 8              # cores
PAD = 64           # numerator band half-width; max class size must be <= 65
FP8_SCALE = 16.0   # pre-quantization scale (keeps fp8 e4m3 out of denormals)
CH = 512           # qt chunk width
NCH = N // CH      # 16
ACT_SCALE = float(1.0 / (TEMP * FP8_SCALE * FP8_SCALE))

# out tile layout (f32 columns)
A_OFF = 0          # [8 x 8] telescoped per-(rowtile, chunk-slot) E_acc row sums
DS_OFF = 64        # [8] diag-tile row sums (f32 exp)
ED_OFF = 72        # [8] exact diagonal exp
N1_OFF = 80        # [8] numerator gate accum, diag tile
N2_OFF = 88        # [8] numerator gate accum, +64 band
D32_OFF = 96       # [8] d=32 tile row sums
COL_OFF = 104      # [64 den colsums][8 num colsums]
OUTW = 176

_AXON_SO = "/opt/axon/libaxon_pjrt.so"

LAST_RESULTS = None   # BassKernelResults of the most recent run (for test.py)


def _install_axon_trace_hook():
    """Provide antenv.axon_hooks (NTFF profiling) if the image lacks it."""
    try:
        from antenv.axon_hooks import get_axon_ntff_profile_hook  # noqa: F401
        return
    except ImportError:
        pass
    if not os.path.exists(_AXON_SO):
        return
    try:
        lib = ctypes.CDLL(_AXON_SO)
    except OSError:
        return
    if not hasattr(lib, "axon_start_nrt_profile"):
        return
    lib.axon_start_nrt_profile.argtypes = [ctypes.POINTER(ctypes.c_int64), ctypes.c_size_t]
    lib.axon_start_nrt_profile.restype = ctypes.c_int64
    lib.axon_stop_nrt_profile.argtypes = [ctypes.c_char_p]
    lib.axon_stop_nrt_profile.restype = ctypes.c_int64

    @contextlib.contextmanager
    def _hook(output_dir, device_ids):
        import jax
        jax.devices()
        if device_ids:
            ids = (ctypes.c_int64 * len(device_ids))(*device_ids)
            rc = lib.axon_start_nrt_profile(ids, len(device_ids))
        else:
            rc = lib.axon_start_nrt_profile(None, 0)
        if rc != 0:
            raise RuntimeError(f"axon_start_nrt_profile rc={rc}")
        try:
            yield
        finally:
            n = lib.axon_stop_nrt_profile(str(output_dir).encode())
            if n < 0:
                raise RuntimeError(f"axon_stop_nrt_profile rc={n}")

    _the_hook = [_hook]
    mod = types.ModuleType("antenv.axon_hooks")
    mod.set_axon_ntff_profile_hook = lambda h: _the_hook.__setitem__(0, h)
    mod.get_axon_ntff_profile_hook = lambda: _the_hook[0]
    sys.modules["antenv.axon_hooks"] = mod
    import antenv
    antenv.axon_hooks = mod


def _split_excess_waits(nc, max_waits=1):
    """This walrus build allows one sync-wait per instruction; move extras
    onto same-engine NoOps inserted just before (execution order preserved)."""
    for f in nc.m.functions:
        for b in f.blocks:
            insts = b.instructions
            new = []
            changed = False
            for inst in insts:
                si = inst.sync_info
                ow = list(si.on_wait) if (si and si.on_wait) else []
                if len(ow) > max_waits:
                    extra, keep = ow[:-max_waits], ow[-max_waits:]
                    for k, w in enumerate(extra):
                        nop = mybir.InstNoOp(name=f"{inst.name}-w{k}", ins=[], outs=[])
                        nop.engine = inst.engine
                        nop.sync_info = mybir.SyncInfo(on_wait=[w], on_update=[])
                        new.append(nop)
                    inst.sync_info = mybir.SyncInfo(
                        on_wait=keep,
                        on_update=list(si.on_update) if si.on_update else [])
                    changed = True
                new.append(inst)
            if changed:
                b.instructions = new


# last contributing row-tile index k for each local column tile jt
def _last_contrib():
    last = {}
    for jt in range(64):
        ks = [k for k in range(8) if (jt - 8 * k) % 64 in range(1, 32)]
        last[jt] = max(ks)
    return last


def _build_nc():
    f32 = mybir.dt.float32
    bf16 = mybir.dt.bfloat16
    fp16 = mybir.dt.float16
    fp8 = mybir.dt.float8e4
    Alu = mybir.AluOpType
    Act = mybir.ActivationFunctionType
    DR = mybir.MatmulPerfMode.DoubleRow

    nc = bass.Bass(trn_type="TRN2", target_bir_lowering=False, debug=False)
    qT = nc.dram_tensor("qT", [128, NCH * 4 * CH], fp8, kind="ExternalInput")
    labwd = nc.dram_tensor("labw", [8 * 192], fp16, kind="ExternalInput")
    lrowd = nc.dram_tensor("lrow", [128, 8], fp16, kind="ExternalInput")
    antid = nc.dram_tensor("anti", [128, 128], f32, kind="ExternalInput")
    identd = nc.dram_tensor("ident", [128, 128], f32, kind="ExternalInput")
    outd = nc.dram_tensor("out", [128, OUTW], f32, kind="ExternalOutput")

    last_contrib = _last_contrib()

    with tile.TileContext(nc) as tc, contextlib.ExitStack() as ctx:
        qp = ctx.enter_context(tc.tile_pool(name="qp", bufs=1))
        pp = ctx.enter_context(tc.tile_pool(name="pp", bufs=2, space="PSUM"))
        cpp = ctx.enter_context(tc.tile_pool(name="cpp", bufs=1, space="PSUM"))
        ep = ctx.enter_context(tc.tile_pool(name="ep", bufs=3))
        dp = ctx.enter_context(tc.tile_pool(name="dp", bufs=2))
        wp = ctx.enter_context(tc.tile_pool(name="wp", bufs=2))
        sp = ctx.enter_context(tc.tile_pool(name="sp", bufs=1))

        # ---- preload ----
        qt = []
        for n in range(NCH):
            q = qp.tile([128, 4, CH], fp8, tag=f"q{n}")
            nc.sync.dma_start(out=q, in_=qT[:, n * 4 * CH:(n + 1) * 4 * CH])
            qt.append(q)
        labw = sp.tile([128, 8, 192], fp16)
        nc.sync.dma_start(
            out=labw, in_=bass.AP(tensor=labwd, offset=0, ap=[[0, 128], [1, 8 * 192]]))
        lrow = sp.tile([128, 8], fp16)
        nc.sync.dma_start(out=lrow, in_=lrowd.ap())
        anti = sp.tile([128, 128], f32)
        nc.sync.dma_start(out=anti, in_=antid.ap())
        ident = sp.tile([128, 128], f32)
        nc.sync.dma_start(out=ident, in_=identd.ap())
        ones = sp.tile([128, 1], bf16)
        nc.vector.memset(ones, 1.0)
        warm = sp.tile([128, 128], bf16)
        nc.vector.memset(warm, 0.0)
        warm_ps = pp.tile([128, 3 * CH], f32, tag="ps")
        for w in range(48):
            nc.tensor.matmul(warm_ps[:, :128], warm, warm, start=True, stop=True)

        # E_acc chunk tiles (bf16), zeroed on gpsimd while input DMA streams
        eacc = []
        for n in range(NCH):
            t_ = sp.tile([128, CH], bf16, tag=f"eacc{n}")
            nc.gpsimd.memset(t_, 0.0)
            eacc.append(t_)

        out_t = sp.tile([128, OUTW], f32)
        colps = cpp.tile([128, 72], f32, tag="cps")

        # ---- main loop over local row-tiles L = 8k ----
        for k in range(8):
            r0 = 1024 * k
            a = 2 * k              # lhsT chunk (r0 % 512 == 0)
            scr_g0 = None
            for g in range(3):
                width = 1152 if g == 2 else 1536
                ps = pp.tile([128, 3 * CH], f32, tag="ps")
                # fill psum: chunk pieces (full 512s; d=32 piece is 128 wide)
                for j in range(3):
                    n = (2 * k + 3 * g + j) % NCH
                    po = j * CH
                    w = 128 if (g == 2 and j == 2) else CH
                    for kk in (0, 2):
                        nc.tensor.matmul(
                            ps[:, po:po + w],
                            qt[a][:, kk:kk + 2, 0:128],
                            qt[n][:, kk:kk + 2, 0:w],
                            start=(kk == 0), stop=(kk == 2),
                            perf_mode=DR)
                scr = ep.tile([128, 3 * CH], bf16, tag="scr")
                if g == 0:
                    # diag tile: f32 exp (exact self-term cancellation in den)
                    ediag = dp.tile([128, 128], f32, tag="ed")
                    nc.scalar.activation(out=ediag, in_=ps[:, 0:128],
                                         func=Act.Exp, scale=ACT_SCALE)
                    nc.vector.memset(scr[:, 0:128], 0.0)
                    nc.scalar.activation(out=scr[:, 128:1536],
                                         in_=ps[:, 128:1536],
                                         func=Act.Exp, scale=ACT_SCALE)
                    scr_g0 = scr
                else:
                    nc.scalar.activation(out=scr[:, 0:width],
                                         in_=ps[:, 0:width],
                                         func=Act.Exp, scale=ACT_SCALE)
                if g == 0:
                    # ---- per-row-tile extras off the diag tile ----
                    w0 = wp.tile([128, 128], f32, tag="w0")
                    nc.vector.tensor_scalar(
                        out=w0, in0=ediag,
                        scalar1=1.0, scalar2=0.0, op0=Alu.mult, op1=Alu.add,
                        accum_out=out_t[:, DS_OFF + k:DS_OFF + k + 1])
                    u1 = wp.tile([128, 128], f32, tag="u1")
                    nc.vector.scalar_tensor_tensor(
                        out=u1, in0=labw[:, k, 0:128], scalar=lrow[:, k:k + 1],
                        in1=ediag, op0=Alu.is_equal, op1=Alu.mult)
                    w2 = wp.tile([128, 128], f32, tag="w2")
                    nc.vector.scalar_tensor_tensor(
                        out=w2, in0=u1, scalar=1.0,
                        in1=ident, op0=Alu.mult, op1=Alu.mult,
                        accum_out=out_t[:, ED_OFF + k:ED_OFF + k + 1])
                    nc.vector.tensor_tensor(out=u1, in0=u1, in1=anti, op=Alu.mult)
                    w3 = wp.tile([128, 128], f32, tag="w3")
                    nc.vector.scalar_tensor_tensor(
                        out=w3, in0=u1, scalar=1.0,
                        in1=u1, op0=Alu.is_gt, op1=Alu.mult,
                        accum_out=out_t[:, N1_OFF + k:N1_OFF + k + 1])
                    # +64 band: u2 gate -> nacc2; kept for masked colsum
                    u2 = wp.tile([128, PAD], bf16, tag="u2")
                    nc.vector.scalar_tensor_tensor(
                        out=u2, in0=labw[:, k, 128:128 + PAD],
                        scalar=lrow[:, k:k + 1], in1=scr[:, 128:128 + PAD],
                        op0=Alu.is_equal, op1=Alu.mult)
                    u2g = wp.tile([128, PAD], bf16, tag="u2g")
                    nc.vector.scalar_tensor_tensor(
                        out=u2g, in0=u2, scalar=1.0, in1=u2,
                        op0=Alu.is_gt, op1=Alu.mult,
                        accum_out=out_t[:, N2_OFF + k:N2_OFF + k + 1])
                    nc.tensor.matmul(colps[0:PAD, 64 + k:65 + k],
                                     u2g, ones[:, 0:1], start=True, stop=True)
                if g == 2:
                    # d=32 tile row sum (no E_acc add, no colsum)
                    w4 = wp.tile([128, 128], bf16, tag="w4")
                    nc.vector.tensor_scalar(
                        out=w4,
                        in0=scr[:, 1024:1152],
                        scalar1=1.0, scalar2=0.0, op0=Alu.mult, op1=Alu.add,
                        accum_out=out_t[:, D32_OFF + k:D32_OFF + k + 1])
                # E_acc adds, one per full chunk; telescoped row-sum accum
                nch_in_g = 2 if g == 2 else 3
                for j in range(nch_in_g):
                    ji = 3 * g + j            # chunk slot 0..7
                    n = (2 * k + ji) % NCH
                    nc.vector.scalar_tensor_tensor(
                        out=eacc[n], in0=scr[:, j * CH:(j + 1) * CH], scalar=1.0,
                        in1=eacc[n], op0=Alu.mult, op1=Alu.add,
                        accum_out=out_t[:, A_OFF + k * 8 + ji:A_OFF + k * 8 + ji + 1])
            # colsums for tiles whose last contributor is row-tile k
            for jt in range(64):
                if last_contrib[jt] != k:
                    continue
                n = jt // 4
                jl = jt % 4
                nc.tensor.matmul(
                    colps[:, jt:jt + 1],
                    eacc[n][:, jl * 128:(jl + 1) * 128],
                    ones[:, 0:1], start=True, stop=True)

        nc.vector.tensor_copy(out=out_t[:, COL_OFF:COL_OFF + 72], in_=colps[:, 0:72])
        nc.sync.dma_start(out=outd.ap(), in_=out_t)

    _split_excess_waits(nc)
    return nc


_NC = None


def _get_nc():
    global _NC
    if _NC is None:
        _NC = _build_nc()
    return _NC


def _host_reference(emb, lab):
    """Numpy fallback (only for pathological label distributions where a
    class exceeds the band margin; never triggers for the target regime)."""
    e = emb / np.linalg.norm(emb, axis=1, keepdims=True).astype(np.float32)
    sim = (e @ e.T).astype(np.float32) / np.float32(TEMP)
    E = np.exp(sim, dtype=np.float32)
    pos = (lab[:, None] == lab[None, :]) & ~np.eye(len(lab), dtype=bool)
    valid = pos & (sim > 0)
    num = np.where(valid, E, 0).sum(1, dtype=np.float32)
    den = E.sum(1, dtype=np.float32) - np.diagonal(E)
    rv = valid.any(1) & (den > 0)
    ns = np.where(rv, num, np.float32(1.0))
    ds = np.where(rv, den, np.float32(1.0))
    li = np.log(ds + np.float32(EPS)) - np.log(ns)
    nv = int(rv.sum())
    if nv == 0:
        return np.float32(0.0)
    return np.float32(abs(float(np.where(rv, li, 0).sum(dtype=np.float64)) / nv))


def kernel(**inputs):
    global LAST_RESULTS
    emb = np.ascontiguousarray(np.asarray(inputs["embeddings"], dtype=np.float32))
    lab = np.asarray(inputs["labels"]).astype(np.int64).ravel()
    assert emb.shape == (N, D) and lab.shape == (N,)

    if np.bincount(lab, minlength=1).max() > PAD + 1:
        return _host_reference(emb, lab)

    _install_axon_trace_hook()

    # host prep: normalize, sort by label, per-core roll, quantize fp8
    e = emb / np.linalg.norm(emb, axis=1, keepdims=True).astype(np.float32)
    order = np.argsort(lab, kind="stable")
    es = np.ascontiguousarray(e[order])
    ls = lab[order]
    lf = ls.astype(np.float16)

    anti = (1.0 - np.eye(128, dtype=np.float32)).astype(np.float32)
    ident = np.eye(128, dtype=np.float32)

    in_maps = []
    for c in range(M):
        roll = c * 128
        q = np.roll(es, -roll, axis=0)
        lq = np.roll(lf, -roll)
        qTc = ((q.T * np.float32(FP8_SCALE))
               .reshape(4, 128, NCH, CH)
               .transpose(1, 2, 0, 3)
               .reshape(128, NCH * 4 * CH)
               .astype(ml_dtypes.float8_e4m3))
        labwin = np.empty((8, 192), dtype=np.float16)
        lrw = np.empty((128, 8), dtype=np.float16)
        for k in range(8):
            r0 = 1024 * k
            idx = (np.arange(r0, r0 + 192)) % N
            labwin[k] = lq[idx]
            lrw[:, k] = lq[r0:r0 + 128]
        in_maps.append({
            "qT": np.ascontiguousarray(qTc),
            "labw": np.ascontiguousarray(labwin.reshape(-1)),
            "lrow": np.ascontiguousarray(lrw),
            "anti": anti,
            "ident": ident,
        })

    nc = _get_nc()
    res = run_bass_kernel_spmd(nc, in_maps, core_ids=list(range(M)))
    LAST_RESULTS = res

    # ---- host assembly (sorted order), then loss ----
    den = np.zeros(N, dtype=np.float64)
    num = np.zeros(N, dtype=np.float64)
    for c in range(M):
        roll = c * 128
        o = res.results[c]["out"].astype(np.float64)   # [128, OUTW]
        prevA = {}
        for k in range(8):
            r0 = 1024 * k
            rowpart = np.zeros(128, dtype=np.float64)
            for ji in range(8):
                ch = (2 * k + ji) % NCH
                cur = o[:, A_OFF + k * 8 + ji]
                rowpart += cur - prevA.get(ch, 0.0)
                prevA[ch] = cur
            rows = (np.arange(r0, r0 + 128) + roll) % N
            den[rows] += rowpart + o[:, DS_OFF + k] + o[:, D32_OFF + k] - o[:, ED_OFF + k]
            num[rows] += o[:, N1_OFF + k] + o[:, N2_OFF + k]
            band = (np.arange(r0 + 128, r0 + 128 + PAD) + roll) % N
            num[band] += o[:PAD, COL_OFF + 64 + k]
        for jt in range(64):
            cols = (np.arange(jt * 128, jt * 128 + 128) + roll) % N
            den[cols] += o[:, COL_OFF + jt]

    rv = (num > 0) & (den > 0)
    nv = int(rv.sum())
    if nv == 0:
        return np.float32(0.0)
    ns = np.where(rv, num, 1.0)
    ds = np.where(rv, den, 1.0)
    li = np.log(ds + EPS) - np.log(ns)
    return np.float32(abs(li[rv].sum() / nv))
